# revision 9
# baseline (speedup 1.0000x reference)
"""GroupedSwiGLU MoE kernel for 8x Trainium2 NeuronCores.

Strategy: load-balanced expert-parallel. Token counts per expert are
rounded to 128-token units; for the balanced path the unit multiset is
decomposed into sixteen 3-unit and eight 2-unit pieces so every core
runs exactly eight units (1024 tokens) as three slots of (384,384,256)
tokens, each slot carrying its own expert's weights. Inside each core:
  per slot:
    phase 1: gateT/upT[inter, tok] = Wg/Wu^T-contracted matmuls vs xT
    swiglu : hT = silu(gateT) * upT
    phase 2: out[tok, hid] = hT^T-contracted matmuls vs Wd, scaled by probs
All matmul operands bf16 (fp32 PSUM accumulate); host does the
transpose/tiling/padding and the final scatter-gather. Falls back to
the single-slot max-padded program when the decomposition is infeasible.
"""

import numpy as np
import ml_dtypes
from contextlib import ExitStack

import concourse.bass as bass
import concourse.mybir as mybir
import concourse.tile as tile
from concourse.bacc import Bacc
from concourse.bass_utils import run_bass_kernel_spmd

E = 8
HID = 2048
INTER = 1408
P = 128
KO_H = HID // P    # 16 k-tiles for phase-1 contraction
KO_I = INTER // P  # 11 k-tiles for phase-2 contraction / m-tiles in phase 1
NF = 512           # phase-2 moving free chunk (hid)

SLOT_UNITS = (3, 3, 2)   # balanced path: per-core slots in 128-token units
T_BAL = 128 * sum(SLOT_UNITS)
MAXU = max(SLOT_UNITS)

F32 = mybir.dt.float32
BF16 = mybir.dt.bfloat16
NP_BF16 = ml_dtypes.bfloat16

_nc_cache: dict = {}


# ─────────────────────────── balanced program ───────────────────────────

def _build_balanced():
    """Per-core program: 3 slots of (384,384,256) tokens, 1024 total."""
    nc = Bacc()
    S = len(SLOT_UNITS)
    xT = nc.dram_tensor("xT", [P, KO_H, T_BAL], BF16, kind="ExternalInput")
    probs = nc.dram_tensor("probs", [P, T_BAL // P], F32, kind="ExternalInput")
    wg_in = [
        nc.dram_tensor(f"wg{s}", [P, KO_I, KO_H, P], BF16, kind="ExternalInput")
        for s in range(S)
    ]
    wu_in = [
        nc.dram_tensor(f"wu{s}", [P, KO_I, KO_H, P], BF16, kind="ExternalInput")
        for s in range(S)
    ]
    NNF = HID // NF
    wd_in = [
        nc.dram_tensor(f"wd{s}", [P, NNF, KO_I, NF], BF16, kind="ExternalInput")
        for s in range(S)
    ]
    out = nc.dram_tensor("out", [T_BAL, HID], BF16, kind="ExternalOutput")

    with tile.TileContext(nc) as tc, ExitStack() as ctx:
        resident = ctx.enter_context(tc.tile_pool(name="resident", bufs=1))
        wdpool = ctx.enter_context(tc.tile_pool(name="wd", bufs=4))
        wpool = ctx.enter_context(tc.tile_pool(name="weights", bufs=8))
        hpool = ctx.enter_context(tc.tile_pool(name="h", bufs=2))
        tmp = ctx.enter_context(tc.tile_pool(name="tmp", bufs=3))
        opool = ctx.enter_context(tc.tile_pool(name="outp", bufs=4))
        psum = ctx.enter_context(tc.tile_pool(name="psum", bufs=2, space="PSUM"))
        psum2 = ctx.enter_context(tc.tile_pool(name="psum2", bufs=4, space="PSUM"))

        slot_off = []
        o = 0
        for su in SLOT_UNITS:
            slot_off.append(o)
            o += su * P

        # xT slot 0 first so the first phase-1 m-tile only waits on slot 0's
        # token columns; later slots stream in mid-phase-1 (see below).
        xT_sb = resident.tile([P, KO_H, T_BAL], BF16)
        for k in range(KO_H):
            nc.sync.dma_start(
                xT_sb[:, k, : SLOT_UNITS[0] * P], xT[:, k, : SLOT_UNITS[0] * P]
            )
        probs_dma = resident.tile([P, T_BAL // P], F32)
        nc.sync.dma_start(probs_dma[:], probs[:])
        # Bounce through DVE so phase-2 scaling (DVE) only ever needs the PE
        # wait: the TensorScalar ISA slot can't carry a second (DMA) wait.
        probs_sb = resident.tile([P, T_BAL // P], F32)
        nc.vector.tensor_copy(probs_sb[:], probs_dma[:])

        for s in range(S):
            Ts = SLOT_UNITS[s] * P
            toff = slot_off[s]
            # wd for this slot, streamed in [KO_I, 512]-column slices on the
            # scalar queue; slices 0-1 issue late in phase 1, 2-3 inside
            # phase 2, spreading HBM demand away from the wg/wu burst.
            wd_n = [
                wdpool.tile([P, KO_I, NF], BF16, tag="wdn", name=f"wdn{s}_{i}")
                for i in range(NNF)
            ]
            hT = hpool.tile([P, KO_I, MAXU * P], BF16, tag="h")

            # Phase 1: per inter m-tile, gateT/upT psum then fused silu*mul
            for m in range(KO_I):
                if m == 1 and s + 1 < S:
                    so = slot_off[s + 1]
                    sw = SLOT_UNITS[s + 1] * P
                    for k in range(KO_H):
                        nc.sync.dma_start(
                            xT_sb[:, k, so : so + sw], xT[:, k, so : so + sw]
                        )
                if m in (7, 9):
                    n = (m - 7) // 2
                    nc.scalar.dma_start(wd_n[n][:], wd_in[s][:, n])
                wg_m = wpool.tile([P, KO_H, P], BF16, tag="wg")
                nc.gpsimd.dma_start(wg_m[:], wg_in[s][:, m])
                wu_m = wpool.tile([P, KO_H, P], BF16, tag="wu")
                nc.gpsimd.dma_start(wu_m[:], wu_in[s][:, m])
                pg = psum.tile([P, NF], F32, tag="pg")
                pu = psum.tile([P, NF], F32, tag="pu")
                for k in range(KO_H):
                    nc.tensor.matmul(
                        pg[:, :Ts], wg_m[:, k], xT_sb[:, k, toff : toff + Ts],
                        start=(k == 0), stop=(k == KO_H - 1),
                    )
                for k in range(KO_H):
                    nc.tensor.matmul(
                        pu[:, :Ts], wu_m[:, k], xT_sb[:, k, toff : toff + Ts],
                        start=(k == 0), stop=(k == KO_H - 1),
                    )
                sg = tmp.tile([P, MAXU * P], F32, tag="sg")
                nc.scalar.activation(
                    sg[:, :Ts], pg[:, :Ts], mybir.ActivationFunctionType.Silu
                )
                # ACT copy of up-psum so the DVE mul has a single-engine wait
                su = tmp.tile([P, MAXU * P], F32, tag="su")
                nc.scalar.copy(su[:, :Ts], pu[:, :Ts])
                nc.vector.tensor_mul(hT[:, m, :Ts], sg[:, :Ts], su[:, :Ts])

            # Phase 2: out tiles [128 tok, 512 hid], contract over inter.
            # n-outer so each wd slice is consumed right after it lands.
            for n in range(NNF):
                if n + 2 < NNF:
                    nc.scalar.dma_start(wd_n[n + 2][:], wd_in[s][:, n + 2])
                for t in range(SLOT_UNITS[s]):
                    g = toff // P + t
                    po = psum2.tile([P, NF], F32, tag="po")
                    for k in range(KO_I):
                        nc.tensor.matmul(
                            po[:], hT[:, k, bass.ts(t, P)],
                            wd_n[n][:, k],
                            start=(k == 0), stop=(k == KO_I - 1),
                        )
                    ot = opool.tile([P, NF], BF16, tag="ot")
                    nc.vector.tensor_scalar_mul(ot[:], po[:], probs_sb[:, g : g + 1])
                    nc.sync.dma_start(out[bass.ts(g, P), bass.ts(n, NF)], ot[:])
    nc.finalize()
    return nc


def _decompose_332(units):
    """Split each unit count into 3s and 2s with exactly 16 threes total."""
    opts = []
    for u in units:
        o = [(a, (u - 3 * a) // 2) for a in range(u // 3 + 1) if (u - 3 * a) % 2 == 0]
        if not o:
            return None
        opts.append(o)
    reach = {0: []}
    for o in opts:
        nr = {}
        for ssum, path in reach.items():
            for ab in o:
                ns = ssum + ab[0]
                if ns <= 16 and ns not in nr:
                    nr[ns] = path + [ab]
        reach = nr
    return reach.get(16)


def _pieces(counts):
    """Per-core slot assignment [(expert, unit_offset) x 3] or None."""
    u = [(int(c) + P - 1) // P for c in counts]
    U = sum(u)
    if U > 64:
        return None
    units = list(u)
    experts = list(range(len(counts)))
    if U < 64:
        units.append(64 - U)
        experts.append(-1)  # dummy: zero data
    dec = _decompose_332(units)
    if dec is None:
        return None
    threes, twos = [], []
    for e, (a, b) in zip(experts, dec):
        off = 0
        for _ in range(a):
            threes.append((e, off))
            off += 3
        for _ in range(b):
            twos.append((e, off))
            off += 2
    if len(threes) != 16 or len(twos) != 8:
        return None
    return [[threes[2 * i], threes[2 * i + 1], twos[i]] for i in range(E)]


# ─────────────────────────── host-side packing ───────────────────────────

def _tile_w1(w):
    """[HID, INTER] -> [P, KO_I, KO_H, P] bf16 (gate/up layout)."""
    return np.ascontiguousarray(
        w.reshape(KO_H, P, KO_I, P).transpose(1, 2, 0, 3)
    ).astype(NP_BF16)


def _tile_wd(w):
    """[INTER, HID] -> [P, KO_I, HID] bf16 (down layout, fallback)."""
    return np.ascontiguousarray(
        w.reshape(KO_I, P, HID).transpose(1, 0, 2)
    ).astype(NP_BF16)


def _tile_wd4(w):
    """[INTER, HID] -> [P, HID//NF, KO_I, NF] bf16 (sliced down layout)."""
    return np.ascontiguousarray(
        w.reshape(KO_I, P, HID // NF, NF).transpose(1, 2, 0, 3)
    ).astype(NP_BF16)


def _tile_x(x_pad, T):
    """[T, HID] -> [P, KO_H, T] bf16."""
    return np.ascontiguousarray(
        x_pad.T.reshape(KO_H, P, T).transpose(1, 0, 2)
    ).astype(NP_BF16)


def _tile_probs(p_pad, T):
    """[T] -> [P, T//P] f32."""
    return np.ascontiguousarray(p_pad.reshape(T // P, P).T).astype(np.float32)


def _run_balanced(x, probs, wg, wu, wd, counts, offs, cores, trace):
    if "bal" not in _nc_cache:
        _nc_cache["bal"] = _build_balanced()
    nc = _nc_cache["bal"]

    wg_t = {}
    wu_t = {}
    wd_t = {}
    for e in set(e for core in cores for (e, _) in core):
        if e < 0:
            wg_t[e] = np.zeros((P, KO_I, KO_H, P), NP_BF16)
            wu_t[e] = wg_t[e]
            wd_t[e] = np.zeros((P, HID // NF, KO_I, NF), NP_BF16)
        else:
            wg_t[e] = _tile_w1(wg[e])
            wu_t[e] = _tile_w1(wu[e])
            wd_t[e] = _tile_wd4(wd[e])

    # token ranges per piece: piece (e, uoff) covers padded-expert tokens
    # [uoff*128, (uoff+su)*128); real rows are the first counts[e]-uoff*128.
    in_maps = []
    piece_rows = []  # per core: list of (global_start, n_real, local_start)
    for core in cores:
        x_core = np.zeros((T_BAL, HID), np.float32)
        p_core = np.zeros((T_BAL,), np.float32)
        rows = []
        lo = 0
        m = {}
        for s, (e, uoff) in enumerate(core):
            su = SLOT_UNITS[s]
            if e >= 0:
                gs = int(offs[e]) + uoff * P
                n_real = max(0, min(int(counts[e]) - uoff * P, su * P))
                if n_real > 0:
                    x_core[lo : lo + n_real] = x[gs : gs + n_real]
                    p_core[lo : lo + n_real] = probs[gs : gs + n_real]
                rows.append((gs, n_real, lo))
            m[f"wg{s}"] = wg_t[e]
            m[f"wu{s}"] = wu_t[e]
            m[f"wd{s}"] = wd_t[e]
            lo += su * P
        m["xT"] = _tile_x(x_core, T_BAL)
        m["probs"] = _tile_probs(p_core, T_BAL)
        in_maps.append(m)
        piece_rows.append(rows)

    res = run_bass_kernel_spmd(nc, in_maps, core_ids=list(range(E)), trace=trace)

    y = np.empty((x.shape[0], HID), np.float32)
    for c in range(E):
        o = np.asarray(res.results[c]["out"]).astype(np.float32)
        for gs, n_real, lo in piece_rows[c]:
            if n_real > 0:
                y[gs : gs + n_real] = o[lo : lo + n_real]
    return y, res


# ─────────────────── fallback: single-slot max-padded ───────────────────

def _build_single(T: int):
    """Per-core Bass program for T padded tokens (T % 512 == 0)."""
    TF = 512
    nc = Bacc()
    xT = nc.dram_tensor("xT", [P, KO_H, T], BF16, kind="ExternalInput")
    wg = nc.dram_tensor("wg", [P, KO_I, KO_H, P], BF16, kind="ExternalInput")
    wu = nc.dram_tensor("wu", [P, KO_I, KO_H, P], BF16, kind="ExternalInput")
    wd = nc.dram_tensor("wd", [P, KO_I, HID], BF16, kind="ExternalInput")
    probs = nc.dram_tensor("probs", [P, T // P], F32, kind="ExternalInput")
    out = nc.dram_tensor("out", [T, HID], F32, kind="ExternalOutput")

    n_tf = T // TF
    n_t = T // P
    n_nf = HID // NF

    with tile.TileContext(nc) as tc, ExitStack() as ctx:
        resident = ctx.enter_context(tc.tile_pool(name="resident", bufs=1))
        wpool = ctx.enter_context(tc.tile_pool(name="weights", bufs=2))
        tmp = ctx.enter_context(tc.tile_pool(name="tmp", bufs=3))
        opool = ctx.enter_context(tc.tile_pool(name="outp", bufs=4))
        psum = ctx.enter_context(tc.tile_pool(name="psum", bufs=2, space="PSUM"))

        xT_sb = resident.tile([P, KO_H, T], BF16)
        for k in range(KO_H):
            nc.sync.dma_start(xT_sb[:, k], xT[:, k])
        wd_sb = resident.tile([P, KO_I, HID], BF16)
        for k in range(KO_I):
            nc.sync.dma_start(wd_sb[:, k], wd[:, k])
        probs_dma = resident.tile([P, T // P], F32)
        nc.sync.dma_start(probs_dma[:], probs[:])
        probs_sb = resident.tile([P, T // P], F32)
        nc.vector.tensor_copy(probs_sb[:], probs_dma[:])
        hT_sb = resident.tile([P, KO_I, T], BF16)

        for m in range(KO_I):
            wg_m = wpool.tile([P, KO_H, P], BF16, tag="wg")
            nc.gpsimd.dma_start(wg_m[:], wg[:, m])
            wu_m = wpool.tile([P, KO_H, P], BF16, tag="wu")
            nc.gpsimd.dma_start(wu_m[:], wu[:, m])
            for f in range(n_tf):
                pg = psum.tile([P, TF], F32, tag="pg")
                pu = psum.tile([P, TF], F32, tag="pu")
                for k in range(KO_H):
                    nc.tensor.matmul(
                        pg[:], wg_m[:, k], xT_sb[:, k, bass.ts(f, TF)],
                        start=(k == 0), stop=(k == KO_H - 1),
                    )
                for k in range(KO_H):
                    nc.tensor.matmul(
                        pu[:], wu_m[:, k], xT_sb[:, k, bass.ts(f, TF)],
                        start=(k == 0), stop=(k == KO_H - 1),
                    )
                sg = tmp.tile([P, TF], F32, tag="sg")
                nc.scalar.activation(
                    sg[:], pg[:], mybir.ActivationFunctionType.Silu
                )
                su = tmp.tile([P, TF], F32, tag="su")
                nc.scalar.copy(su[:], pu[:])
                nc.vector.tensor_mul(
                    hT_sb[:, m, bass.ts(f, TF)], sg[:], su[:]
                )

        for t in range(n_t):
            for n in range(n_nf):
                po = psum.tile([P, NF], F32, tag="po")
                for k in range(KO_I):
                    nc.tensor.matmul(
                        po[:], hT_sb[:, k, bass.ts(t, P)],
                        wd_sb[:, k, bass.ts(n, NF)],
                        start=(k == 0), stop=(k == KO_I - 1),
                    )
                ot = opool.tile([P, NF], F32, tag="ot")
                nc.vector.tensor_scalar_mul(ot[:], po[:], probs_sb[:, t : t + 1])
                nc.sync.dma_start(out[bass.ts(t, P), bass.ts(n, NF)], ot[:])
    nc.finalize()
    return nc


def _run_single(x, probs, wg, wu, wd, counts, offs, trace):
    T = int(max(1, counts.max()))
    T = ((T + 511) // 512) * 512

    key = ("single", T)
    if key not in _nc_cache:
        _nc_cache[key] = _build_single(T)
    nc = _nc_cache[key]

    in_maps = []
    for e in range(E):
        n = int(counts[e])
        s = int(offs[e])
        x_pad = np.zeros((T, HID), np.float32)
        x_pad[:n] = x[s : s + n]
        p_pad = np.zeros((T,), np.float32)
        p_pad[:n] = probs[s : s + n]
        in_maps.append(
            {
                "xT": _tile_x(x_pad, T),
                "wg": _tile_w1(wg[e]),
                "wu": _tile_w1(wu[e]),
                "wd": _tile_wd(wd[e]),
                "probs": _tile_probs(p_pad, T),
            }
        )

    res = run_bass_kernel_spmd(nc, in_maps, core_ids=list(range(E)), trace=trace)

    y = np.empty((x.shape[0], HID), np.float32)
    for e in range(E):
        n = int(counts[e])
        s = int(offs[e])
        y[s : s + n] = res.results[e]["out"][:n]
    return y, res


def _run(inputs, trace=False):
    x = np.asarray(inputs["permuted_x"], np.float32)
    probs = np.asarray(inputs["permuted_probs"], np.float32)
    wg = np.asarray(inputs["w_gate"], np.float32)
    wu = np.asarray(inputs["w_up"], np.float32)
    wd = np.asarray(inputs["w_down"], np.float32)
    counts = np.asarray(inputs["tokens_per_expert"]).astype(np.int64)
    offs = np.concatenate([[0], np.cumsum(counts)])
    assert offs[-1] == x.shape[0]

    cores = _pieces(counts)
    if cores is not None:
        return _run_balanced(x, probs, wg, wu, wd, counts, offs, cores, trace)
    return _run_single(x, probs, wg, wu, wd, counts, offs, trace)


def kernel(**inputs) -> np.ndarray:
    y, _ = _run(inputs, trace=False)
    return y


# revision 10
# speedup vs baseline: 1.0282x; 1.0282x over previous
"""GroupedSwiGLU MoE kernel for 8x Trainium2 NeuronCores.

Strategy: load-balanced expert-parallel. Token counts per expert are
rounded to 128-token units; for the balanced path the unit multiset is
decomposed into sixteen 3-unit and eight 2-unit pieces so every core
runs exactly eight units (1024 tokens) as three slots of (384,384,256)
tokens, each slot carrying its own expert's weights. Inside each core:
  per slot:
    phase 1: gateT/upT[inter, tok] = Wg/Wu^T-contracted matmuls vs xT
    swiglu : hT = silu(gateT) * upT
    phase 2: out[tok, hid] = hT^T-contracted matmuls vs Wd, scaled by probs
All matmul operands bf16 (fp32 PSUM accumulate); host does the
transpose/tiling/padding and the final scatter-gather. Falls back to
the single-slot max-padded program when the decomposition is infeasible.
"""

import numpy as np
import ml_dtypes
from contextlib import ExitStack

import concourse.bass as bass
import concourse.mybir as mybir
import concourse.tile as tile
from concourse.bacc import Bacc
from concourse.bass_utils import run_bass_kernel_spmd

E = 8
HID = 2048
INTER = 1408
P = 128
KO_H = HID // P    # 16 k-tiles for phase-1 contraction
KO_I = INTER // P  # 11 k-tiles for phase-2 contraction / m-tiles in phase 1
NF = 512           # phase-2 moving free chunk (hid)

SLOT_UNITS = (3, 3, 2)   # balanced path: per-core slots in 128-token units
T_BAL = 128 * sum(SLOT_UNITS)
MAXU = max(SLOT_UNITS)

F32 = mybir.dt.float32
BF16 = mybir.dt.bfloat16
NP_BF16 = ml_dtypes.bfloat16

_nc_cache: dict = {}


# ─────────────────────────── balanced program ───────────────────────────

def _build_balanced():
    """Per-core program: 3 slots of (384,384,256) tokens, 1024 total."""
    nc = Bacc()
    S = len(SLOT_UNITS)
    xT = nc.dram_tensor("xT", [P, KO_H, T_BAL], BF16, kind="ExternalInput")
    probs = nc.dram_tensor("probs", [P, T_BAL // P], F32, kind="ExternalInput")
    wg_in = [
        nc.dram_tensor(f"wg{s}", [P, KO_I, KO_H, P], BF16, kind="ExternalInput")
        for s in range(S)
    ]
    wu_in = [
        nc.dram_tensor(f"wu{s}", [P, KO_I, KO_H, P], BF16, kind="ExternalInput")
        for s in range(S)
    ]
    NNF = HID // NF
    wd_in = [
        nc.dram_tensor(f"wd{s}", [P, NNF, KO_I, NF], BF16, kind="ExternalInput")
        for s in range(S)
    ]
    out = nc.dram_tensor("out", [T_BAL, HID], BF16, kind="ExternalOutput")

    with tile.TileContext(nc) as tc, ExitStack() as ctx:
        resident = ctx.enter_context(tc.tile_pool(name="resident", bufs=1))
        wdpool = ctx.enter_context(tc.tile_pool(name="wd", bufs=6))
        wpool = ctx.enter_context(tc.tile_pool(name="weights", bufs=8))
        hpool = ctx.enter_context(tc.tile_pool(name="h", bufs=2))
        tmp = ctx.enter_context(tc.tile_pool(name="tmp", bufs=3))
        opool = ctx.enter_context(tc.tile_pool(name="outp", bufs=3))
        psum = ctx.enter_context(tc.tile_pool(name="psum", bufs=2, space="PSUM"))
        psum2 = ctx.enter_context(tc.tile_pool(name="psum2", bufs=4, space="PSUM"))

        slot_off = []
        o = 0
        for su in SLOT_UNITS:
            slot_off.append(o)
            o += su * P

        # xT slot 0 up-front on sync; slots 1-2 stream from the scalar
        # engine mid-phase-1 (scalar is serialized behind silu work, so
        # those transfers genuinely defer past the startup HBM crunch).
        xT_sb = resident.tile([P, KO_H, T_BAL], BF16)
        for k in range(KO_H):
            nc.sync.dma_start(
                xT_sb[:, k, : SLOT_UNITS[0] * P], xT[:, k, : SLOT_UNITS[0] * P]
            )
        probs_dma = resident.tile([P, T_BAL // P], F32)
        nc.sync.dma_start(probs_dma[:], probs[:])
        # Bounce through DVE so phase-2 scaling (DVE) only ever needs the PE
        # wait: the TensorScalar ISA slot can't carry a second (DMA) wait.
        probs_sb = resident.tile([P, T_BAL // P], F32)
        nc.vector.tensor_copy(probs_sb[:], probs_dma[:])

        wd_tiles: dict = {}

        def ensure_wd(s):
            if s not in wd_tiles:
                wd_tiles[s] = [
                    wdpool.tile([P, KO_I, NF], BF16, tag="wdn", name=f"wdn{s}_{i}")
                    for i in range(NNF)
                ]

        def wd_dma(s, n):
            ensure_wd(s)
            nc.scalar.dma_start(wd_tiles[s][n][:], wd_in[s][:, n])

        def xT_slot_dma(s):
            so = slot_off[s]
            sw = SLOT_UNITS[s] * P
            for k in range(KO_H):
                nc.scalar.dma_start(
                    xT_sb[:, k, so : so + sw], xT[:, k, so : so + sw]
                )

        for s in range(S):
            Ts = SLOT_UNITS[s] * P
            toff = slot_off[s]
            ensure_wd(s)
            hT = hpool.tile([P, KO_I, MAXU * P], BF16, tag="h")

            # Phase 1: per inter m-tile, gateT/upT psum then fused silu*mul
            for m in range(KO_I):
                wg_m = wpool.tile([P, KO_H, P], BF16, tag="wg")
                wu_m = wpool.tile([P, KO_H, P], BF16, tag="wu")
                if s == 0 and m == 0:
                    # halve the first weight transfers so the very first
                    # matmuls start sooner
                    h2 = KO_H // 2
                    nc.gpsimd.dma_start(wg_m[:, :h2], wg_in[s][:, m, :h2])
                    nc.gpsimd.dma_start(wg_m[:, h2:], wg_in[s][:, m, h2:])
                    nc.gpsimd.dma_start(wu_m[:, :h2], wu_in[s][:, m, :h2])
                    nc.gpsimd.dma_start(wu_m[:, h2:], wu_in[s][:, m, h2:])
                else:
                    nc.gpsimd.dma_start(wg_m[:], wg_in[s][:, m])
                    nc.gpsimd.dma_start(wu_m[:], wu_in[s][:, m])
                pg = psum.tile([P, NF], F32, tag="pg")
                pu = psum.tile([P, NF], F32, tag="pu")
                for k in range(KO_H):
                    nc.tensor.matmul(
                        pg[:, :Ts], wg_m[:, k], xT_sb[:, k, toff : toff + Ts],
                        start=(k == 0), stop=(k == KO_H - 1),
                    )
                for k in range(KO_H):
                    nc.tensor.matmul(
                        pu[:, :Ts], wu_m[:, k], xT_sb[:, k, toff : toff + Ts],
                        start=(k == 0), stop=(k == KO_H - 1),
                    )
                sg = tmp.tile([P, MAXU * P], F32, tag="sg")
                nc.scalar.activation(
                    sg[:, :Ts], pg[:, :Ts], mybir.ActivationFunctionType.Silu
                )
                # ACT copy of up-psum so the DVE mul has a single-engine wait
                su = tmp.tile([P, MAXU * P], F32, tag="su")
                nc.scalar.copy(su[:, :Ts], pu[:, :Ts])
                nc.vector.tensor_mul(hT[:, m, :Ts], sg[:, :Ts], su[:, :Ts])

                # Deferred transfers, serialized behind this m-tile's silu
                # on the scalar engine stream:
                if s == 0:
                    if m == 1:
                        wd_dma(0, 0)
                    elif m == 2:
                        xT_slot_dma(1)
                    elif m == 3:
                        wd_dma(0, 1)
                    elif m == 6:
                        xT_slot_dma(2)
                if m == 5:
                    wd_dma(s, 2)
                elif m == 7:
                    wd_dma(s, 3)
                elif m == KO_I - 1 and s + 1 < S:
                    wd_dma(s + 1, 0)
                    wd_dma(s + 1, 1)

            # Phase 2: per token tile, 4 hid chunks into one SBUF tile,
            # then a single 512KB output DMA (fewer DMAs -> shorter BSP
            # epilogue).
            for t in range(SLOT_UNITS[s]):
                g = toff // P + t
                ot = opool.tile([P, HID], BF16, tag="ot")
                for n in range(NNF):
                    po = psum2.tile([P, NF], F32, tag="po")
                    for k in range(KO_I):
                        nc.tensor.matmul(
                            po[:], hT[:, k, bass.ts(t, P)],
                            wd_tiles[s][n][:, k],
                            start=(k == 0), stop=(k == KO_I - 1),
                        )
                    nc.vector.tensor_scalar_mul(
                        ot[:, bass.ts(n, NF)], po[:], probs_sb[:, g : g + 1]
                    )
                nc.sync.dma_start(out[bass.ts(g, P)], ot[:])
            del wd_tiles[s]
    nc.finalize()
    return nc


def _decompose_332(units):
    """Split each unit count into 3s and 2s with exactly 16 threes total."""
    opts = []
    for u in units:
        o = [(a, (u - 3 * a) // 2) for a in range(u // 3 + 1) if (u - 3 * a) % 2 == 0]
        if not o:
            return None
        opts.append(o)
    reach = {0: []}
    for o in opts:
        nr = {}
        for ssum, path in reach.items():
            for ab in o:
                ns = ssum + ab[0]
                if ns <= 16 and ns not in nr:
                    nr[ns] = path + [ab]
        reach = nr
    return reach.get(16)


def _pieces(counts):
    """Per-core slot assignment [(expert, unit_offset) x 3] or None."""
    u = [(int(c) + P - 1) // P for c in counts]
    U = sum(u)
    if U > 64:
        return None
    units = list(u)
    experts = list(range(len(counts)))
    if U < 64:
        units.append(64 - U)
        experts.append(-1)  # dummy: zero data
    dec = _decompose_332(units)
    if dec is None:
        return None
    threes, twos = [], []
    for e, (a, b) in zip(experts, dec):
        off = 0
        for _ in range(a):
            threes.append((e, off))
            off += 3
        for _ in range(b):
            twos.append((e, off))
            off += 2
    if len(threes) != 16 or len(twos) != 8:
        return None
    return [[threes[2 * i], threes[2 * i + 1], twos[i]] for i in range(E)]


# ─────────────────────────── host-side packing ───────────────────────────

def _tile_w1(w):
    """[HID, INTER] -> [P, KO_I, KO_H, P] bf16 (gate/up layout)."""
    return np.ascontiguousarray(
        w.reshape(KO_H, P, KO_I, P).transpose(1, 2, 0, 3)
    ).astype(NP_BF16)


def _tile_wd(w):
    """[INTER, HID] -> [P, KO_I, HID] bf16 (down layout, fallback)."""
    return np.ascontiguousarray(
        w.reshape(KO_I, P, HID).transpose(1, 0, 2)
    ).astype(NP_BF16)


def _tile_wd4(w):
    """[INTER, HID] -> [P, HID//NF, KO_I, NF] bf16 (sliced down layout)."""
    return np.ascontiguousarray(
        w.reshape(KO_I, P, HID // NF, NF).transpose(1, 2, 0, 3)
    ).astype(NP_BF16)


def _tile_x(x_pad, T):
    """[T, HID] -> [P, KO_H, T] bf16."""
    return np.ascontiguousarray(
        x_pad.T.reshape(KO_H, P, T).transpose(1, 0, 2)
    ).astype(NP_BF16)


def _tile_probs(p_pad, T):
    """[T] -> [P, T//P] f32."""
    return np.ascontiguousarray(p_pad.reshape(T // P, P).T).astype(np.float32)


def _run_balanced(x, probs, wg, wu, wd, counts, offs, cores, trace):
    if "bal" not in _nc_cache:
        _nc_cache["bal"] = _build_balanced()
    nc = _nc_cache["bal"]

    wg_t = {}
    wu_t = {}
    wd_t = {}
    for e in set(e for core in cores for (e, _) in core):
        if e < 0:
            wg_t[e] = np.zeros((P, KO_I, KO_H, P), NP_BF16)
            wu_t[e] = wg_t[e]
            wd_t[e] = np.zeros((P, HID // NF, KO_I, NF), NP_BF16)
        else:
            wg_t[e] = _tile_w1(wg[e])
            wu_t[e] = _tile_w1(wu[e])
            wd_t[e] = _tile_wd4(wd[e])

    # token ranges per piece: piece (e, uoff) covers padded-expert tokens
    # [uoff*128, (uoff+su)*128); real rows are the first counts[e]-uoff*128.
    in_maps = []
    piece_rows = []  # per core: list of (global_start, n_real, local_start)
    for core in cores:
        x_core = np.zeros((T_BAL, HID), np.float32)
        p_core = np.zeros((T_BAL,), np.float32)
        rows = []
        lo = 0
        m = {}
        for s, (e, uoff) in enumerate(core):
            su = SLOT_UNITS[s]
            if e >= 0:
                gs = int(offs[e]) + uoff * P
                n_real = max(0, min(int(counts[e]) - uoff * P, su * P))
                if n_real > 0:
                    x_core[lo : lo + n_real] = x[gs : gs + n_real]
                    p_core[lo : lo + n_real] = probs[gs : gs + n_real]
                rows.append((gs, n_real, lo))
            m[f"wg{s}"] = wg_t[e]
            m[f"wu{s}"] = wu_t[e]
            m[f"wd{s}"] = wd_t[e]
            lo += su * P
        m["xT"] = _tile_x(x_core, T_BAL)
        m["probs"] = _tile_probs(p_core, T_BAL)
        in_maps.append(m)
        piece_rows.append(rows)

    res = run_bass_kernel_spmd(nc, in_maps, core_ids=list(range(E)), trace=trace)

    y = np.empty((x.shape[0], HID), np.float32)
    for c in range(E):
        o = np.asarray(res.results[c]["out"]).astype(np.float32)
        for gs, n_real, lo in piece_rows[c]:
            if n_real > 0:
                y[gs : gs + n_real] = o[lo : lo + n_real]
    return y, res


# ─────────────────── fallback: single-slot max-padded ───────────────────

def _build_single(T: int):
    """Per-core Bass program for T padded tokens (T % 512 == 0)."""
    TF = 512
    nc = Bacc()
    xT = nc.dram_tensor("xT", [P, KO_H, T], BF16, kind="ExternalInput")
    wg = nc.dram_tensor("wg", [P, KO_I, KO_H, P], BF16, kind="ExternalInput")
    wu = nc.dram_tensor("wu", [P, KO_I, KO_H, P], BF16, kind="ExternalInput")
    wd = nc.dram_tensor("wd", [P, KO_I, HID], BF16, kind="ExternalInput")
    probs = nc.dram_tensor("probs", [P, T // P], F32, kind="ExternalInput")
    out = nc.dram_tensor("out", [T, HID], F32, kind="ExternalOutput")

    n_tf = T // TF
    n_t = T // P
    n_nf = HID // NF

    with tile.TileContext(nc) as tc, ExitStack() as ctx:
        resident = ctx.enter_context(tc.tile_pool(name="resident", bufs=1))
        wpool = ctx.enter_context(tc.tile_pool(name="weights", bufs=2))
        tmp = ctx.enter_context(tc.tile_pool(name="tmp", bufs=3))
        opool = ctx.enter_context(tc.tile_pool(name="outp", bufs=4))
        psum = ctx.enter_context(tc.tile_pool(name="psum", bufs=2, space="PSUM"))

        xT_sb = resident.tile([P, KO_H, T], BF16)
        for k in range(KO_H):
            nc.sync.dma_start(xT_sb[:, k], xT[:, k])
        wd_sb = resident.tile([P, KO_I, HID], BF16)
        for k in range(KO_I):
            nc.sync.dma_start(wd_sb[:, k], wd[:, k])
        probs_dma = resident.tile([P, T // P], F32)
        nc.sync.dma_start(probs_dma[:], probs[:])
        probs_sb = resident.tile([P, T // P], F32)
        nc.vector.tensor_copy(probs_sb[:], probs_dma[:])
        hT_sb = resident.tile([P, KO_I, T], BF16)

        for m in range(KO_I):
            wg_m = wpool.tile([P, KO_H, P], BF16, tag="wg")
            nc.gpsimd.dma_start(wg_m[:], wg[:, m])
            wu_m = wpool.tile([P, KO_H, P], BF16, tag="wu")
            nc.gpsimd.dma_start(wu_m[:], wu[:, m])
            for f in range(n_tf):
                pg = psum.tile([P, TF], F32, tag="pg")
                pu = psum.tile([P, TF], F32, tag="pu")
                for k in range(KO_H):
                    nc.tensor.matmul(
                        pg[:], wg_m[:, k], xT_sb[:, k, bass.ts(f, TF)],
                        start=(k == 0), stop=(k == KO_H - 1),
                    )
                for k in range(KO_H):
                    nc.tensor.matmul(
                        pu[:], wu_m[:, k], xT_sb[:, k, bass.ts(f, TF)],
                        start=(k == 0), stop=(k == KO_H - 1),
                    )
                sg = tmp.tile([P, TF], F32, tag="sg")
                nc.scalar.activation(
                    sg[:], pg[:], mybir.ActivationFunctionType.Silu
                )
                su = tmp.tile([P, TF], F32, tag="su")
                nc.scalar.copy(su[:], pu[:])
                nc.vector.tensor_mul(
                    hT_sb[:, m, bass.ts(f, TF)], sg[:], su[:]
                )

        for t in range(n_t):
            for n in range(n_nf):
                po = psum.tile([P, NF], F32, tag="po")
                for k in range(KO_I):
                    nc.tensor.matmul(
                        po[:], hT_sb[:, k, bass.ts(t, P)],
                        wd_sb[:, k, bass.ts(n, NF)],
                        start=(k == 0), stop=(k == KO_I - 1),
                    )
                ot = opool.tile([P, NF], F32, tag="ot")
                nc.vector.tensor_scalar_mul(ot[:], po[:], probs_sb[:, t : t + 1])
                nc.sync.dma_start(out[bass.ts(t, P), bass.ts(n, NF)], ot[:])
    nc.finalize()
    return nc


def _run_single(x, probs, wg, wu, wd, counts, offs, trace):
    T = int(max(1, counts.max()))
    T = ((T + 511) // 512) * 512

    key = ("single", T)
    if key not in _nc_cache:
        _nc_cache[key] = _build_single(T)
    nc = _nc_cache[key]

    in_maps = []
    for e in range(E):
        n = int(counts[e])
        s = int(offs[e])
        x_pad = np.zeros((T, HID), np.float32)
        x_pad[:n] = x[s : s + n]
        p_pad = np.zeros((T,), np.float32)
        p_pad[:n] = probs[s : s + n]
        in_maps.append(
            {
                "xT": _tile_x(x_pad, T),
                "wg": _tile_w1(wg[e]),
                "wu": _tile_w1(wu[e]),
                "wd": _tile_wd(wd[e]),
                "probs": _tile_probs(p_pad, T),
            }
        )

    res = run_bass_kernel_spmd(nc, in_maps, core_ids=list(range(E)), trace=trace)

    y = np.empty((x.shape[0], HID), np.float32)
    for e in range(E):
        n = int(counts[e])
        s = int(offs[e])
        y[s : s + n] = res.results[e]["out"][:n]
    return y, res


def _run(inputs, trace=False):
    x = np.asarray(inputs["permuted_x"], np.float32)
    probs = np.asarray(inputs["permuted_probs"], np.float32)
    wg = np.asarray(inputs["w_gate"], np.float32)
    wu = np.asarray(inputs["w_up"], np.float32)
    wd = np.asarray(inputs["w_down"], np.float32)
    counts = np.asarray(inputs["tokens_per_expert"]).astype(np.int64)
    offs = np.concatenate([[0], np.cumsum(counts)])
    assert offs[-1] == x.shape[0]

    cores = _pieces(counts)
    if cores is not None:
        return _run_balanced(x, probs, wg, wu, wd, counts, offs, cores, trace)
    return _run_single(x, probs, wg, wu, wd, counts, offs, trace)


def kernel(**inputs) -> np.ndarray:
    y, _ = _run(inputs, trace=False)
    return y


# revision 18
# speedup vs baseline: 1.0530x; 1.0241x over previous
"""GroupedSwiGLU MoE kernel for 8x Trainium2 NeuronCores.

Strategy: load-balanced expert-parallel. Token counts per expert are
rounded to 128-token units; for the balanced path the unit multiset is
decomposed into sixteen 3-unit and eight 2-unit pieces so every core
runs exactly eight units (1024 tokens) as three slots of (384,384,256)
tokens, each slot carrying its own expert's weights. Inside each core:
  per slot:
    phase 1: gateT/upT[inter, tok] = Wg/Wu^T-contracted matmuls vs xT
    swiglu : hT = silu(gateT) * upT
    phase 2: out[tok, hid] = hT^T-contracted matmuls vs Wd, scaled by probs
All matmul operands bf16 (fp32 PSUM accumulate); host does the
transpose/tiling/padding and the final scatter-gather. Falls back to
the single-slot max-padded program when the decomposition is infeasible.
"""

import numpy as np
import ml_dtypes
from contextlib import ExitStack

import concourse.bass as bass
import concourse.mybir as mybir
import concourse.tile as tile
from concourse.bacc import Bacc
from concourse.bass_utils import run_bass_kernel_spmd

E = 8
HID = 2048
INTER = 1408
P = 128
KO_H = HID // P    # 16 k-tiles for phase-1 contraction
KO_I = INTER // P  # 11 k-tiles for phase-2 contraction / m-tiles in phase 1
NF = 512           # phase-2 moving free chunk (hid)

SLOT_UNITS = (3, 3, 2)   # balanced path: per-core slots in 128-token units
T_BAL = 128 * sum(SLOT_UNITS)
MAXU = max(SLOT_UNITS)

F32 = mybir.dt.float32
BF16 = mybir.dt.bfloat16
NP_BF16 = ml_dtypes.bfloat16

_nc_cache: dict = {}


# ─────────────────────────── balanced program ───────────────────────────

def _build_balanced():
    """Per-core program: 3 slots of (384,384,256) tokens, 1024 total."""
    nc = Bacc()
    S = len(SLOT_UNITS)
    probs = nc.dram_tensor("probs", [P, T_BAL // P], F32, kind="ExternalInput")
    wg_in = [
        nc.dram_tensor(f"wg{s}", [P, KO_I, KO_H, P], BF16, kind="ExternalInput")
        for s in range(S)
    ]
    wu_in = [
        nc.dram_tensor(f"wu{s}", [P, KO_I, KO_H, P], BF16, kind="ExternalInput")
        for s in range(S)
    ]
    NNF = HID // NF
    wd_in = [
        nc.dram_tensor(f"wd{s}", [P, NNF, KO_I, NF], BF16, kind="ExternalInput")
        for s in range(S)
    ]
    xT_in = [
        nc.dram_tensor(
            f"xTs{s}", [P, KO_H, SLOT_UNITS[s] * P], BF16, kind="ExternalInput"
        )
        for s in range(S)
    ]
    out = nc.dram_tensor("out", [T_BAL, HID], BF16, kind="ExternalOutput")

    with tile.TileContext(nc) as tc, ExitStack() as ctx:
        resident = ctx.enter_context(tc.tile_pool(name="resident", bufs=1))
        wdpool = ctx.enter_context(tc.tile_pool(name="wd", bufs=6))
        wpool = ctx.enter_context(tc.tile_pool(name="weights", bufs=8))
        hpool = ctx.enter_context(tc.tile_pool(name="h", bufs=2))
        tmp = ctx.enter_context(tc.tile_pool(name="tmp", bufs=3))
        opool = ctx.enter_context(tc.tile_pool(name="outp", bufs=3))
        psum = ctx.enter_context(tc.tile_pool(name="psum", bufs=2, space="PSUM"))
        psum2 = ctx.enter_context(tc.tile_pool(name="psum2", bufs=4, space="PSUM"))

        slot_off = []
        o = 0
        for su in SLOT_UNITS:
            slot_off.append(o)
            o += su * P

        # Per-slot xT tiles (contiguous per partition -> 4KB DMA packets).
        # Slot 0 up-front, split across the sync and scalar rings; slots 1-2
        # stream from the scalar engine mid-phase-1 (scalar is serialized
        # behind silu work, so those transfers genuinely defer past the
        # startup HBM crunch).
        xs = [
            resident.tile(
                [P, KO_H, SLOT_UNITS[s] * P], BF16, name=f"xs{s}"
            )
            for s in range(S)
        ]
        h2 = KO_H // 2
        nc.sync.dma_start(xs[0][:, :h2], xT_in[0][:, :h2])
        nc.scalar.dma_start(xs[0][:, h2:], xT_in[0][:, h2:])
        probs_dma = resident.tile([P, T_BAL // P], F32)
        nc.sync.dma_start(probs_dma[:], probs[:])
        # Bounce through DVE so phase-2 scaling (DVE) only ever needs the PE
        # wait: the TensorScalar ISA slot can't carry a second (DMA) wait.
        probs_sb = resident.tile([P, T_BAL // P], F32)
        nc.vector.tensor_copy(probs_sb[:], probs_dma[:])

        wd_tiles: dict = {}

        def ensure_wd(s):
            if s not in wd_tiles:
                wd_tiles[s] = [
                    wdpool.tile([P, KO_I, NF], BF16, tag="wdn", name=f"wdn{s}_{i}")
                    for i in range(NNF)
                ]

        def wd_dma(s, n):
            ensure_wd(s)
            nc.scalar.dma_start(wd_tiles[s][n][:], wd_in[s][:, n])

        for s in range(S):
            Ts = SLOT_UNITS[s] * P
            toff = slot_off[s]
            ensure_wd(s)
            hT = hpool.tile([P, KO_I, MAXU * P], BF16, tag="h")

            # Phase 1: per inter m-tile, gateT/upT psum then fused silu*mul.
            # wg streams on the gpsimd ring, wu on the sync ring: two rings
            # double the weight-delivery bandwidth the PE sees.
            for m in range(KO_I):
                wg_m = wpool.tile([P, KO_H, P], BF16, tag="wg")
                wu_m = wpool.tile([P, KO_H, P], BF16, tag="wu")
                if s == 0 and m == 0:
                    # halve the first weight transfers so the very first
                    # matmuls start sooner
                    nc.gpsimd.dma_start(wg_m[:, :h2], wg_in[s][:, m, :h2])
                    nc.gpsimd.dma_start(wg_m[:, h2:], wg_in[s][:, m, h2:])
                    nc.sync.dma_start(wu_m[:, :h2], wu_in[s][:, m, :h2])
                    nc.sync.dma_start(wu_m[:, h2:], wu_in[s][:, m, h2:])
                else:
                    nc.gpsimd.dma_start(wg_m[:], wg_in[s][:, m])
                    nc.sync.dma_start(wu_m[:], wu_in[s][:, m])
                pg = psum.tile([P, NF], F32, tag="pg")
                pu = psum.tile([P, NF], F32, tag="pu")
                for k in range(KO_H):
                    nc.tensor.matmul(
                        pg[:, :Ts], wg_m[:, k], xs[s][:, k],
                        start=(k == 0), stop=(k == KO_H - 1),
                    )
                for k in range(KO_H):
                    nc.tensor.matmul(
                        pu[:, :Ts], wu_m[:, k], xs[s][:, k],
                        start=(k == 0), stop=(k == KO_H - 1),
                    )
                sg = tmp.tile([P, MAXU * P], F32, tag="sg")
                nc.scalar.activation(
                    sg[:, :Ts], pg[:, :Ts], mybir.ActivationFunctionType.Silu
                )
                # ACT copy of up-psum so the DVE mul has a single-engine wait
                su = tmp.tile([P, MAXU * P], F32, tag="su")
                nc.scalar.copy(su[:, :Ts], pu[:, :Ts])
                nc.vector.tensor_mul(hT[:, m, :Ts], sg[:, :Ts], su[:, :Ts])

                # Deferred transfers, serialized behind this m-tile's silu
                # on the scalar engine stream:
                if s == 0:
                    if m == 1:
                        wd_dma(0, 0)
                    elif m == 2:
                        nc.scalar.dma_start(xs[1][:], xT_in[1][:])
                    elif m == 3:
                        wd_dma(0, 1)
                    elif m == 6:
                        nc.scalar.dma_start(xs[2][:], xT_in[2][:])
                if m == 5:
                    wd_dma(s, 2)
                elif m == 7:
                    wd_dma(s, 3)
                elif m == KO_I - 1 and s + 1 < S:
                    wd_dma(s + 1, 0)
                    wd_dma(s + 1, 1)

            # Phase 2: per token tile, 4 hid chunks into one SBUF tile,
            # then a single 512KB output DMA (fewer DMAs -> shorter BSP
            # epilogue).
            for t in range(SLOT_UNITS[s]):
                g = toff // P + t
                ot = opool.tile([P, HID], BF16, tag="ot")
                for n in range(NNF):
                    po = psum2.tile([P, NF], F32, tag="po")
                    for k in range(KO_I):
                        nc.tensor.matmul(
                            po[:], hT[:, k, bass.ts(t, P)],
                            wd_tiles[s][n][:, k],
                            start=(k == 0), stop=(k == KO_I - 1),
                        )
                    nc.vector.tensor_scalar_mul(
                        ot[:, bass.ts(n, NF)], po[:], probs_sb[:, g : g + 1]
                    )
                nc.scalar.dma_start(out[bass.ts(g, P)], ot[:])
            del wd_tiles[s]
    nc.finalize()
    return nc


def _decompose_332(units):
    """Split each unit count into 3s and 2s with exactly 16 threes total."""
    opts = []
    for u in units:
        o = [(a, (u - 3 * a) // 2) for a in range(u // 3 + 1) if (u - 3 * a) % 2 == 0]
        if not o:
            return None
        opts.append(o)
    reach = {0: []}
    for o in opts:
        nr = {}
        for ssum, path in reach.items():
            for ab in o:
                ns = ssum + ab[0]
                if ns <= 16 and ns not in nr:
                    nr[ns] = path + [ab]
        reach = nr
    return reach.get(16)


def _pieces(counts):
    """Per-core slot assignment [(expert, unit_offset) x 3] or None."""
    u = [(int(c) + P - 1) // P for c in counts]
    U = sum(u)
    if U > 64:
        return None
    units = list(u)
    experts = list(range(len(counts)))
    if U < 64:
        units.append(64 - U)
        experts.append(-1)  # dummy: zero data
    dec = _decompose_332(units)
    if dec is None:
        return None
    threes, twos = [], []
    for e, (a, b) in zip(experts, dec):
        off = 0
        for _ in range(a):
            threes.append((e, off))
            off += 3
        for _ in range(b):
            twos.append((e, off))
            off += 2
    if len(threes) != 16 or len(twos) != 8:
        return None
    return [[threes[2 * i], threes[2 * i + 1], twos[i]] for i in range(E)]


# ─────────────────────────── host-side packing ───────────────────────────

def _tile_w1(w):
    """[HID, INTER] -> [P, KO_I, KO_H, P] bf16 (gate/up layout)."""
    return np.ascontiguousarray(
        w.reshape(KO_H, P, KO_I, P).transpose(1, 2, 0, 3)
    ).astype(NP_BF16)


def _tile_wd(w):
    """[INTER, HID] -> [P, KO_I, HID] bf16 (down layout, fallback)."""
    return np.ascontiguousarray(
        w.reshape(KO_I, P, HID).transpose(1, 0, 2)
    ).astype(NP_BF16)


def _tile_wd4(w):
    """[INTER, HID] -> [P, HID//NF, KO_I, NF] bf16 (sliced down layout)."""
    return np.ascontiguousarray(
        w.reshape(KO_I, P, HID // NF, NF).transpose(1, 2, 0, 3)
    ).astype(NP_BF16)


def _tile_x(x_pad, T):
    """[T, HID] -> [P, KO_H, T] bf16."""
    return np.ascontiguousarray(
        x_pad.T.reshape(KO_H, P, T).transpose(1, 0, 2)
    ).astype(NP_BF16)


def _tile_probs(p_pad, T):
    """[T] -> [P, T//P] f32."""
    return np.ascontiguousarray(p_pad.reshape(T // P, P).T).astype(np.float32)


def _run_balanced(x, probs, wg, wu, wd, counts, offs, cores, trace):
    if "bal" not in _nc_cache:
        _nc_cache["bal"] = _build_balanced()
    nc = _nc_cache["bal"]

    wg_t = {}
    wu_t = {}
    wd_t = {}
    for e in set(e for core in cores for (e, _) in core):
        if e < 0:
            wg_t[e] = np.zeros((P, KO_I, KO_H, P), NP_BF16)
            wu_t[e] = wg_t[e]
            wd_t[e] = np.zeros((P, HID // NF, KO_I, NF), NP_BF16)
        else:
            wg_t[e] = _tile_w1(wg[e])
            wu_t[e] = _tile_w1(wu[e])
            wd_t[e] = _tile_wd4(wd[e])

    # token ranges per piece: piece (e, uoff) covers padded-expert tokens
    # [uoff*128, (uoff+su)*128); real rows are the first counts[e]-uoff*128.
    in_maps = []
    piece_rows = []  # per core: list of (global_start, n_real, local_start)
    for core in cores:
        x_core = np.zeros((T_BAL, HID), np.float32)
        p_core = np.zeros((T_BAL,), np.float32)
        rows = []
        lo = 0
        m = {}
        for s, (e, uoff) in enumerate(core):
            su = SLOT_UNITS[s]
            if e >= 0:
                gs = int(offs[e]) + uoff * P
                n_real = max(0, min(int(counts[e]) - uoff * P, su * P))
                if n_real > 0:
                    x_core[lo : lo + n_real] = x[gs : gs + n_real]
                    p_core[lo : lo + n_real] = probs[gs : gs + n_real]
                rows.append((gs, n_real, lo))
            m[f"wg{s}"] = wg_t[e]
            m[f"wu{s}"] = wu_t[e]
            m[f"wd{s}"] = wd_t[e]
            m[f"xTs{s}"] = _tile_x(x_core[lo : lo + su * P], su * P)
            lo += su * P
        m["probs"] = _tile_probs(p_core, T_BAL)
        in_maps.append(m)
        piece_rows.append(rows)

    res = run_bass_kernel_spmd(nc, in_maps, core_ids=list(range(E)), trace=trace)

    y = np.empty((x.shape[0], HID), np.float32)
    for c in range(E):
        o = np.asarray(res.results[c]["out"]).astype(np.float32)
        for gs, n_real, lo in piece_rows[c]:
            if n_real > 0:
                y[gs : gs + n_real] = o[lo : lo + n_real]
    return y, res


# ─────────────────── fallback: single-slot max-padded ───────────────────

def _build_single(T: int):
    """Per-core Bass program for T padded tokens (T % 512 == 0)."""
    TF = 512
    nc = Bacc()
    xT = nc.dram_tensor("xT", [P, KO_H, T], BF16, kind="ExternalInput")
    wg = nc.dram_tensor("wg", [P, KO_I, KO_H, P], BF16, kind="ExternalInput")
    wu = nc.dram_tensor("wu", [P, KO_I, KO_H, P], BF16, kind="ExternalInput")
    wd = nc.dram_tensor("wd", [P, KO_I, HID], BF16, kind="ExternalInput")
    probs = nc.dram_tensor("probs", [P, T // P], F32, kind="ExternalInput")
    out = nc.dram_tensor("out", [T, HID], F32, kind="ExternalOutput")

    n_tf = T // TF
    n_t = T // P
    n_nf = HID // NF

    with tile.TileContext(nc) as tc, ExitStack() as ctx:
        resident = ctx.enter_context(tc.tile_pool(name="resident", bufs=1))
        wpool = ctx.enter_context(tc.tile_pool(name="weights", bufs=2))
        tmp = ctx.enter_context(tc.tile_pool(name="tmp", bufs=3))
        opool = ctx.enter_context(tc.tile_pool(name="outp", bufs=4))
        psum = ctx.enter_context(tc.tile_pool(name="psum", bufs=2, space="PSUM"))

        xT_sb = resident.tile([P, KO_H, T], BF16)
        for k in range(KO_H):
            nc.sync.dma_start(xT_sb[:, k], xT[:, k])
        wd_sb = resident.tile([P, KO_I, HID], BF16)
        for k in range(KO_I):
            nc.sync.dma_start(wd_sb[:, k], wd[:, k])
        probs_dma = resident.tile([P, T // P], F32)
        nc.sync.dma_start(probs_dma[:], probs[:])
        probs_sb = resident.tile([P, T // P], F32)
        nc.vector.tensor_copy(probs_sb[:], probs_dma[:])
        hT_sb = resident.tile([P, KO_I, T], BF16)

        for m in range(KO_I):
            wg_m = wpool.tile([P, KO_H, P], BF16, tag="wg")
            nc.gpsimd.dma_start(wg_m[:], wg[:, m])
            wu_m = wpool.tile([P, KO_H, P], BF16, tag="wu")
            nc.gpsimd.dma_start(wu_m[:], wu[:, m])
            for f in range(n_tf):
                pg = psum.tile([P, TF], F32, tag="pg")
                pu = psum.tile([P, TF], F32, tag="pu")
                for k in range(KO_H):
                    nc.tensor.matmul(
                        pg[:], wg_m[:, k], xT_sb[:, k, bass.ts(f, TF)],
                        start=(k == 0), stop=(k == KO_H - 1),
                    )
                for k in range(KO_H):
                    nc.tensor.matmul(
                        pu[:], wu_m[:, k], xT_sb[:, k, bass.ts(f, TF)],
                        start=(k == 0), stop=(k == KO_H - 1),
                    )
                sg = tmp.tile([P, TF], F32, tag="sg")
                nc.scalar.activation(
                    sg[:], pg[:], mybir.ActivationFunctionType.Silu
                )
                su = tmp.tile([P, TF], F32, tag="su")
                nc.scalar.copy(su[:], pu[:])
                nc.vector.tensor_mul(
                    hT_sb[:, m, bass.ts(f, TF)], sg[:], su[:]
                )

        for t in range(n_t):
            for n in range(n_nf):
                po = psum.tile([P, NF], F32, tag="po")
                for k in range(KO_I):
                    nc.tensor.matmul(
                        po[:], hT_sb[:, k, bass.ts(t, P)],
                        wd_sb[:, k, bass.ts(n, NF)],
                        start=(k == 0), stop=(k == KO_I - 1),
                    )
                ot = opool.tile([P, NF], F32, tag="ot")
                nc.vector.tensor_scalar_mul(ot[:], po[:], probs_sb[:, t : t + 1])
                nc.sync.dma_start(out[bass.ts(t, P), bass.ts(n, NF)], ot[:])
    nc.finalize()
    return nc


def _run_single(x, probs, wg, wu, wd, counts, offs, trace):
    T = int(max(1, counts.max()))
    T = ((T + 511) // 512) * 512

    key = ("single", T)
    if key not in _nc_cache:
        _nc_cache[key] = _build_single(T)
    nc = _nc_cache[key]

    in_maps = []
    for e in range(E):
        n = int(counts[e])
        s = int(offs[e])
        x_pad = np.zeros((T, HID), np.float32)
        x_pad[:n] = x[s : s + n]
        p_pad = np.zeros((T,), np.float32)
        p_pad[:n] = probs[s : s + n]
        in_maps.append(
            {
                "xT": _tile_x(x_pad, T),
                "wg": _tile_w1(wg[e]),
                "wu": _tile_w1(wu[e]),
                "wd": _tile_wd(wd[e]),
                "probs": _tile_probs(p_pad, T),
            }
        )

    res = run_bass_kernel_spmd(nc, in_maps, core_ids=list(range(E)), trace=trace)

    y = np.empty((x.shape[0], HID), np.float32)
    for e in range(E):
        n = int(counts[e])
        s = int(offs[e])
        y[s : s + n] = res.results[e]["out"][:n]
    return y, res


def _run(inputs, trace=False):
    x = np.asarray(inputs["permuted_x"], np.float32)
    probs = np.asarray(inputs["permuted_probs"], np.float32)
    wg = np.asarray(inputs["w_gate"], np.float32)
    wu = np.asarray(inputs["w_up"], np.float32)
    wd = np.asarray(inputs["w_down"], np.float32)
    counts = np.asarray(inputs["tokens_per_expert"]).astype(np.int64)
    offs = np.concatenate([[0], np.cumsum(counts)])
    assert offs[-1] == x.shape[0]

    cores = _pieces(counts)
    if cores is not None:
        return _run_balanced(x, probs, wg, wu, wd, counts, offs, cores, trace)
    return _run_single(x, probs, wg, wu, wd, counts, offs, trace)


def kernel(**inputs) -> np.ndarray:
    y, _ = _run(inputs, trace=False)
    return y
